# revision 11
# baseline (speedup 1.0000x reference)
import sys
sys.path.insert(0, '/opt/trn_rl_repo')
import numpy as np

import concourse.bass as bass
import concourse.tile as tile
from concourse import bacc, mybir
from concourse.bass_utils import run_bass_kernel_spmd

# ---------------- problem constants (hardcoded per spec) ----------------
NTOT = 1_000_000          # total elements (X is [2, NTOT])
NCORES = 8
Q = 16                    # quadrature nodes
G = 128 // Q              # element groups packed per partition column
F = 512                   # free-dim elements per group per tile (1 PSUM bank fp32)
EPT = G * F               # elements per tile
NC_ELEM = 131072          # per-core padded element count (divisible by EPT)
T = NC_ELEM // EPT        # tiles per core
STAGE = 128 // G          # tiles per output stage (dense 128-partition staging)
NSTAGES = T // STAGE
CH = NC_ELEM // (128 * F) # chunks in phase-1 [128, CH, F] layout
NPAD = NC_ELEM * NCORES

F32 = mybir.dt.float32
F32R = mybir.dt.float32r
F16 = mybir.dt.float16


def _round_f32r(a: np.ndarray) -> np.ndarray:
    """Round fp32 to fp32r (12-bit mantissa, RNE) on host — matches HW rounding."""
    bits = np.ascontiguousarray(a, dtype=np.float32).view(np.uint32)
    low = bits & np.uint32(0x7FF)
    keep = bits & ~np.uint32(0x7FF)
    up = (low > 0x400) | ((low == 0x400) & (((keep >> np.uint32(11)) & np.uint32(1)) == 1))
    out = keep + up.astype(np.uint32) * np.uint32(0x800)
    return out.view(np.float32)


def _quad_consts():
    nodes, _ = np.polynomial.hermite.hermgauss(Q)
    x16 = nodes.astype(np.float16)
    # recorrect weights for the fp16-rounded nodes (moment matching, Hermite basis)
    V = np.polynomial.hermite.hermvander(x16.astype(np.float64), Q - 1)
    rhs = np.zeros(Q)
    rhs[0] = 1.0
    w = np.linalg.solve(V.T, rhs).astype(np.float16).astype(np.float32)
    x = x16.astype(np.float32)  # z = mu + sqrt(2)*sigma*x_q ; sqrt2 folded into std'
    # expansion lhsT: [2G, 128]; out partition m = g*Q + q
    E = np.zeros((2 * G, 128), dtype=np.float32)
    for m in range(128):
        g, q = divmod(m, Q)
        E[g, m] = 1.0
        E[G + g, m] = x[q]
    # reduction lhsT for slot s: [128, 128] with only cols s*G..s*G+G-1 nonzero;
    # all STAGE of them packed side by side as [128, STAGE*128]
    W = np.zeros((128, STAGE * 128), dtype=np.float32)
    for s in range(STAGE):
        for k in range(128):
            g, q = divmod(k, Q)
            W[k, s * 128 + s * G + g] = w[q]
    return E.astype(np.float16), W.astype(np.float16)


def _dram_ap(t_ap: bass.AP, offset: int, pattern) -> bass.AP:
    return bass.AP(tensor=t_ap.tensor, offset=offset, ap=[list(p) for p in pattern])


def build_graph():
    nc = bacc.Bacc("TRN2", target_bir_lowering=False, debug=False, num_devices=NCORES)
    X = nc.dram_tensor("X", [2, NC_ELEM], F32, kind="ExternalInput").ap()
    EXP = nc.dram_tensor("EXP", [2 * G, 128], F16, kind="ExternalInput").ap()
    RED = nc.dram_tensor("RED", [128, STAGE * 128], F16, kind="ExternalInput").ap()
    OUT = nc.dram_tensor("out", [2, NC_ELEM], F32, kind="ExternalOutput").ap()

    with tile.TileContext(nc) as tc:
        with tc.tile_pool(name="consts", bufs=1) as consts, \
             tc.tile_pool(name="phase1", bufs=1) as ph1, \
             tc.tile_pool(name="rhs", bufs=4) as rpool, \
             tc.tile_pool(name="acts", bufs=4) as apool, \
             tc.tile_pool(name="stage", bufs=2) as spool, \
             tc.tile_pool(name="zps", bufs=2, space="PSUM") as zpool, \
             tc.tile_pool(name="hps", bufs=1, space="PSUM") as hpool, \
             tc.tile_pool(name="mps", bufs=2, space="PSUM") as mpool:

            e_sb = consts.tile([2 * G, 128], F16)
            nc.sync.dma_start(e_sb[:], EXP)
            w_sb = consts.tile([128, STAGE * 128], F16)
            nc.sync.dma_start(w_sb[:], RED)

            # HAM heater: fp16/fp32r matmuls don't register as PE activity, so
            # the clock gate stays at 1.2GHz. Run bf16 matmuls to open it
            # (warmup burst overlaps the input DMA / phase 1), then a small
            # heartbeat each tile keeps the activity window busy.
            BF16 = mybir.dt.bfloat16
            hb_w = consts.tile([128, 512], BF16)
            nc.vector.memset(hb_w[:], 0.001)
            hb_ps = hpool.tile([128, 512], F32)
            for _ in range(10):
                nc.tensor.matmul(hb_ps[:], hb_w[:, 0:128], hb_w[:],
                                 start=True, stop=True, skip_group_check=True)

            # ---- phase 1: load X, convert mu -> fp32r, std' = sqrt(2*var) -> fp32r
            # block b = n // F ; partition p = b % 128 ; chunk c = b // 128
            mu_f = ph1.tile([128, CH, F], F32)
            nc.sync.dma_start(mu_f[:], _dram_ap(X, 0, [[F, 128], [128 * F, CH], [1, F]]))
            var_f = ph1.tile([128, CH, F], F32)
            nc.sync.dma_start(var_f[:], _dram_ap(X, NC_ELEM, [[F, 128], [128 * F, CH], [1, F]]))

            mu_r = ph1.tile([128, CH, F], F16)
            nc.vector.tensor_copy(mu_r[:], mu_f[:])
            stdp_r = ph1.tile([128, CH, F], F16)
            nc.scalar.activation(stdp_r[:], var_f[:], mybir.ActivationFunctionType.Sqrt,
                                 scale=2.0)

            # ---- main loop
            for st in range(NSTAGES):
                m1_stage = mpool.tile([128, F], F32, tag="m1s")
                m2_stage = mpool.tile([128, F], F32, tag="m2s")
                for s in range(STAGE):
                    t = st * STAGE + s
                    src_p = (t % STAGE) * G
                    c = t // STAGE
                    rhs_t = rpool.tile([2 * G, F], F16, tag="rhs")
                    nc.sync.dma_start(rhs_t[0:G, :], mu_r[src_p:src_p + G, c, :])
                    nc.sync.dma_start(rhs_t[G:2 * G, :], stdp_r[src_p:src_p + G, c, :])

                    z_ps = zpool.tile([128, F], F32, tag="z")
                    nc.tensor.matmul(z_ps[:], e_sb[:], rhs_t[:], start=True, stop=True)

                    a_t = apool.tile([128, F], F16, tag="a")
                    nc.scalar.activation(a_t[:], z_ps[:], mybir.ActivationFunctionType.Tanh)
                    a2_t = apool.tile([128, F], F16, tag="a2")
                    nc.vector.tensor_mul(a2_t[:], a_t[:], a_t[:])

                    w_s = w_sb[:, s * 128:(s + 1) * 128]
                    nc.tensor.matmul(m1_stage[:], w_s, a_t[:],
                                     start=(s == 0), stop=(s == STAGE - 1),
                                     skip_group_check=True)
                    nc.tensor.matmul(m2_stage[:], w_s, a2_t[:],
                                     start=(s == 0), stop=(s == STAGE - 1),
                                     skip_group_check=True)
                    nc.tensor.matmul(hb_ps[0:64, 0:64], hb_w[:, 0:64],
                                     a_t[:, 0:64].bitcast(BF16),
                                     start=True, stop=True, skip_group_check=True)

                # epilogue: var = m2 - m1^2 ; write outputs
                m1_sb = spool.tile([128, F], F32, tag="m1sb")
                nc.scalar.copy(m1_sb[:], m1_stage[:])
                sq = spool.tile([128, F], F32, tag="sq")
                nc.vector.tensor_mul(sq[:], m1_sb[:], m1_sb[:])
                var_t = spool.tile([128, F], F32, tag="var")
                nc.vector.tensor_sub(var_t[:], m2_stage[:], sq[:])

                off = st * 128 * F
                nc.sync.dma_start(_dram_ap(OUT, off, [[F, 128], [1, F]]), m1_sb[:])
                nc.sync.dma_start(_dram_ap(OUT, NC_ELEM + off, [[F, 128], [1, F]]), var_t[:])

    nc.finalize()
    return nc


_GRAPH = None

def _get_graph():
    global _GRAPH
    if _GRAPH is None:
        _GRAPH = build_graph()
    return _GRAPH


def kernel(X: np.ndarray) -> np.ndarray:
    assert X.shape == (2, NTOT) and X.dtype == np.float32
    nc = _get_graph()
    E_np, W_np = _quad_consts()

    Xp = np.zeros((2, NPAD), dtype=np.float32)
    Xp[:, :NTOT] = X
    in_maps = []
    for i in range(NCORES):
        shard = np.ascontiguousarray(Xp[:, i * NC_ELEM:(i + 1) * NC_ELEM])
        in_maps.append({"X": shard, "EXP": E_np, "RED": W_np})

    res = run_bass_kernel_spmd(nc, in_maps, core_ids=list(range(NCORES)))
    out = np.concatenate([r["out"] for r in res.results], axis=1)
    return np.ascontiguousarray(out[:, :NTOT])


if __name__ == "__main__":
    rng = np.random.default_rng(0)
    X = rng.random((2, NTOT), dtype=np.float32)
    y = kernel(X)
    print("out shape", y.shape, y.dtype)


# revision 12
# speedup vs baseline: 1.1965x; 1.1965x over previous
import sys
sys.path.insert(0, '/opt/trn_rl_repo')
import numpy as np

import concourse.bass as bass
import concourse.tile as tile
from concourse import bacc, mybir
from concourse.bass_utils import run_bass_kernel_spmd

# ---------------- problem constants (hardcoded per spec) ----------------
NTOT = 1_000_000          # total elements (X is [2, NTOT])
NCORES = 8
Q = 16                    # quadrature nodes
G = 128 // Q              # element groups packed per partition column
F = 512                   # free-dim elements per group per tile (1 PSUM bank fp32)
EPT = G * F               # elements per tile
NC_ELEM = 131072          # per-core padded element count (divisible by EPT)
T = NC_ELEM // EPT        # tiles per core
STAGE = 128 // G          # tiles per output stage (dense 128-partition staging)
NSTAGES = T // STAGE
CH = NC_ELEM // (128 * F) # chunks in phase-1 [128, CH, F] layout
NPAD = NC_ELEM * NCORES

F32 = mybir.dt.float32
F32R = mybir.dt.float32r
F16 = mybir.dt.float16


def _round_f32r(a: np.ndarray) -> np.ndarray:
    """Round fp32 to fp32r (12-bit mantissa, RNE) on host — matches HW rounding."""
    bits = np.ascontiguousarray(a, dtype=np.float32).view(np.uint32)
    low = bits & np.uint32(0x7FF)
    keep = bits & ~np.uint32(0x7FF)
    up = (low > 0x400) | ((low == 0x400) & (((keep >> np.uint32(11)) & np.uint32(1)) == 1))
    out = keep + up.astype(np.uint32) * np.uint32(0x800)
    return out.view(np.float32)


def _quad_consts():
    nodes, _ = np.polynomial.hermite.hermgauss(Q)
    x16 = nodes.astype(np.float16)
    # recorrect weights for the fp16-rounded nodes (moment matching, Hermite basis)
    V = np.polynomial.hermite.hermvander(x16.astype(np.float64), Q - 1)
    rhs = np.zeros(Q)
    rhs[0] = 1.0
    w = np.linalg.solve(V.T, rhs).astype(np.float16).astype(np.float32)
    x = x16.astype(np.float32)  # z = mu + sqrt(2)*sigma*x_q ; sqrt2 folded into std'
    # expansion lhsT: [2G, 128]; out partition m = g*Q + q
    E = np.zeros((2 * G, 128), dtype=np.float32)
    for m in range(128):
        g, q = divmod(m, Q)
        E[g, m] = 1.0
        E[G + g, m] = x[q]
    # reduction lhsT for slot s: [128, 128] with only cols s*G..s*G+G-1 nonzero;
    # all STAGE of them packed side by side as [128, STAGE*128]
    W = np.zeros((128, STAGE * 128), dtype=np.float32)
    for s in range(STAGE):
        for k in range(128):
            g, q = divmod(k, Q)
            W[k, s * 128 + s * G + g] = w[q]
    return E.astype(np.float16), W.astype(np.float16)


def _dram_ap(t_ap: bass.AP, offset: int, pattern) -> bass.AP:
    return bass.AP(tensor=t_ap.tensor, offset=offset, ap=[list(p) for p in pattern])


def build_graph():
    nc = bacc.Bacc("TRN2", target_bir_lowering=False, debug=False, num_devices=NCORES)
    X = nc.dram_tensor("X", [2, NC_ELEM], F32, kind="ExternalInput").ap()
    EXP = nc.dram_tensor("EXP", [2 * G, 128], F16, kind="ExternalInput").ap()
    RED = nc.dram_tensor("RED", [128, STAGE * 128], F16, kind="ExternalInput").ap()
    OUT = nc.dram_tensor("out", [2, NC_ELEM], F32, kind="ExternalOutput").ap()

    with tile.TileContext(nc) as tc:
        with tc.tile_pool(name="consts", bufs=1) as consts, \
             tc.tile_pool(name="phase1", bufs=1) as ph1, \
             tc.tile_pool(name="rhs", bufs=4) as rpool, \
             tc.tile_pool(name="acts", bufs=4) as apool, \
             tc.tile_pool(name="stage", bufs=2) as spool, \
             tc.tile_pool(name="zps", bufs=2, space="PSUM") as zpool, \
             tc.tile_pool(name="hps", bufs=1, space="PSUM") as hpool, \
             tc.tile_pool(name="mps", bufs=2, space="PSUM") as mpool:

            e_sb = consts.tile([2 * G, 128], F16)
            nc.sync.dma_start(e_sb[:], EXP)
            w_sb = consts.tile([128, STAGE * 128], F16)
            nc.sync.dma_start(w_sb[:], RED)

            # HAM heater: fp16/fp32r matmuls don't register as PE activity, so
            # the clock gate stays at 1.2GHz. Run bf16 matmuls to open it
            # (warmup burst overlaps the input DMA / phase 1), then a small
            # heartbeat each tile keeps the activity window busy.
            BF16 = mybir.dt.bfloat16
            hb_w = consts.tile([128, 512], BF16)
            nc.vector.memset(hb_w[:], 0.001)
            hb_ps = hpool.tile([128, 512], F32)
            for _ in range(10):
                nc.tensor.matmul(hb_ps[:], hb_w[:, 0:128], hb_w[:],
                                 start=True, stop=True, skip_group_check=True)

            # ---- phase 1: load X, convert mu -> fp32r, std' = sqrt(2*var) -> fp32r
            # block b = n // F ; partition p = b % 128 ; chunk c = b // 128
            mu_f = ph1.tile([128, CH, F], F32)
            nc.sync.dma_start(mu_f[:], _dram_ap(X, 0, [[F, 128], [128 * F, CH], [1, F]]))
            var_f = ph1.tile([128, CH, F], F32)
            nc.sync.dma_start(var_f[:], _dram_ap(X, NC_ELEM, [[F, 128], [128 * F, CH], [1, F]]))

            mu_r = ph1.tile([128, CH, F], F16)
            nc.vector.tensor_copy(mu_r[:], mu_f[:])
            stdp_r = ph1.tile([128, CH, F], F16)
            nc.scalar.activation(stdp_r[:], var_f[:], mybir.ActivationFunctionType.Sqrt,
                                 scale=2.0)

            # ---- main loop
            for st in range(NSTAGES):
                m1_stage = mpool.tile([128, F], F32, tag="m1s")
                m2_stage = mpool.tile([128, F], F32, tag="m2s")
                for s in range(STAGE):
                    t = st * STAGE + s
                    src_p = (t % STAGE) * G
                    c = t // STAGE
                    rhs_t = rpool.tile([2 * G, F], F16, tag="rhs")
                    nc.sync.dma_start(rhs_t[0:G, :], mu_r[src_p:src_p + G, c, :])
                    nc.sync.dma_start(rhs_t[G:2 * G, :], stdp_r[src_p:src_p + G, c, :])

                    z_ps = zpool.tile([128, F], F32, tag="z")
                    nc.tensor.matmul(z_ps[:], e_sb[:], rhs_t[:], start=True, stop=True)

                    a_t = apool.tile([128, F], F16, tag="a")
                    nc.scalar.activation(a_t[:], z_ps[:], mybir.ActivationFunctionType.Tanh)
                    a2_t = apool.tile([128, F], F16, tag="a2")
                    nc.vector.tensor_mul(a2_t[:], a_t[:], a_t[:])

                    w_s = w_sb[:, s * 128:(s + 1) * 128]
                    nc.tensor.matmul(m1_stage[:], w_s, a_t[:],
                                     start=(s == 0), stop=(s == STAGE - 1),
                                     skip_group_check=True)
                    nc.tensor.matmul(m2_stage[:], w_s, a2_t[:],
                                     start=(s == 0), stop=(s == STAGE - 1),
                                     skip_group_check=True)
                    nc.tensor.matmul(hb_ps[0:64, 0:64], hb_w[0:2 * G, 0:64],
                                     rhs_t[:, 0:64].bitcast(BF16),
                                     start=True, stop=True, skip_group_check=True)

                # epilogue: var = m2 - m1^2 ; write outputs
                m1_sb = spool.tile([128, F], F32, tag="m1sb")
                nc.scalar.copy(m1_sb[:], m1_stage[:])
                sq = spool.tile([128, F], F32, tag="sq")
                nc.vector.tensor_mul(sq[:], m1_sb[:], m1_sb[:])
                var_t = spool.tile([128, F], F32, tag="var")
                nc.vector.tensor_sub(var_t[:], m2_stage[:], sq[:])

                off = st * 128 * F
                nc.sync.dma_start(_dram_ap(OUT, off, [[F, 128], [1, F]]), m1_sb[:])
                nc.sync.dma_start(_dram_ap(OUT, NC_ELEM + off, [[F, 128], [1, F]]), var_t[:])

    nc.finalize()
    return nc


_GRAPH = None

def _get_graph():
    global _GRAPH
    if _GRAPH is None:
        _GRAPH = build_graph()
    return _GRAPH


def kernel(X: np.ndarray) -> np.ndarray:
    assert X.shape == (2, NTOT) and X.dtype == np.float32
    nc = _get_graph()
    E_np, W_np = _quad_consts()

    Xp = np.zeros((2, NPAD), dtype=np.float32)
    Xp[:, :NTOT] = X
    in_maps = []
    for i in range(NCORES):
        shard = np.ascontiguousarray(Xp[:, i * NC_ELEM:(i + 1) * NC_ELEM])
        in_maps.append({"X": shard, "EXP": E_np, "RED": W_np})

    res = run_bass_kernel_spmd(nc, in_maps, core_ids=list(range(NCORES)))
    out = np.concatenate([r["out"] for r in res.results], axis=1)
    return np.ascontiguousarray(out[:, :NTOT])


if __name__ == "__main__":
    rng = np.random.default_rng(0)
    X = rng.random((2, NTOT), dtype=np.float32)
    y = kernel(X)
    print("out shape", y.shape, y.dtype)
